# revision 64
# baseline (speedup 1.0000x reference)
"""Causal single-head attention block on 8 TRN2 NeuronCores.

Reference: Q=x@Wq, K=x@Wk, V=x@Wv; S=Q@K^T (no pre-softmax scaling);
causal mask; P=softmax(S); out=(P@V)/sqrt(64).
Shapes: x [4, 2048, 1024] f32, W* [1024, 64] f32 -> out [4, 2048, 64].

Sharding: 8 cores = 4 batches x 2 interleaved query-tile sets.
Core (b, j) handles global 128-row query tiles {2i+j : i=0..7}.

Key design points (vs naive):
  * x is transposed and cast to fp16 on the host; the device loads xT
    directly (no on-chip transposes of x, no duplicate xq load).
  * For SPMD uniformity, j=1 cores get adjacent 128-column blocks of xT
    swapped so query columns sit at even block positions for all cores.
    Key order within a chunk changes, which is harmless (attention sums
    over keys); the causal mask data (per-core) accounts for it.
  * Attention computed transposed: St[t,q] = K @ Q^T per 128-key block,
    so exp() output E already has keys on partitions -> AV matmul needs
    no transposes at all. Rowsum obtained for free via an extra ones
    column appended to V-natural (col 64), accumulated in the same PSUM.
  * Causal mask is preloaded into PSUM with an identity matmul (PE),
    covering the last two 128-key blocks of the diagonal chunk; block
    counts are 2 (even tiles) / 4 (odd tiles) for every core.
  * fp16 for x/W/Q/K/S path, bf16 for E/V (exp range needs bf16);
    1/sqrt(64)=0.125 folded into Wv on the host. rel_err ~5e-3.
  * Input DMAs are merged into a few large transfers (wkv; rest-of-
    weights; 8 half-chunk loads of [128, 8, 256]) to amortize the
    ~650ns/DMA DGE issue overhead while still pipelining: each K|V
    projection runs in two half passes as its halves land.
  * Chunks are exp'd in pair-groups sharing one [128,1024] PSUM tile:
    one ACT instruction per pair halves the ACT per-inst overhead
    (ACT/exp is the critical engine in the back half).
  * ~30 dummy PE matmuls at t=0 warm the PE p-state during the
    initial DMA window so real matmuls run at full clock.
"""

import sys

import numpy as np

try:  # concourse ships in the TRN container; fall back to its known path
    import concourse  # noqa: F401
except ImportError:
    sys.path.insert(0, "/opt/trn_rl_repo")

B, T, C, DK = 4, 2048, 1024, 64
NCH = [1, 1, 2, 2, 3, 3, 4, 4]   # 512-key chunks per local q-tile (both j)
NDUMMY = 30                       # PE p-state warmup matmuls
NEG = -30000.0                    # fp16-safe mask value

_CACHE = {}


def _build():
    import concourse.bacc as bacc
    import concourse.tile as tile
    import concourse.mybir as mybir

    f32 = mybir.dt.float32
    f16 = mybir.dt.float16
    bf16 = mybir.dt.bfloat16
    Exp = mybir.ActivationFunctionType.Exp
    Copy = mybir.ActivationFunctionType.Copy

    nc = bacc.Bacc("TRN2", target_bir_lowering=False, debug=False,
                   enable_asserts=False, num_devices=8)

    xt_d = nc.dram_tensor("xt", [8, 128, T], f16, kind="ExternalInput").ap()
    wkv_d = nc.dram_tensor("wkv", [128, 1024], f16, kind="ExternalInput").ap()
    w2_d = nc.dram_tensor("w2", [128, 896], f16, kind="ExternalInput").ap()
    y_d = nc.dram_tensor("y", [128, 512], f32, kind="ExternalOutput").ap()

    with tile.TileContext(nc) as tc:
        with (
            tc.tile_pool(name="persist", bufs=1) as pp,
            tc.tile_pool(name="epool", bufs=8) as ep,
            tc.tile_pool(name="small", bufs=4) as smp,
            tc.tile_pool(name="pa", bufs=2, space="PSUM") as pa,
            tc.tile_pool(name="pb", bufs=2, space="PSUM") as pb,
            tc.tile_pool(name="pc", bufs=1, space="PSUM") as pc,
        ):
            warm = pp.tile([128, 128], f16, tag="warm", name="warm")
            wkv = pp.tile([128, 1024], f16, tag="wkv", name="wkv")
            w2 = pp.tile([128, 896], f16, tag="w2", name="w2")
            wq = w2[:, 0:512]
            dmask = w2[:, 512:768]
            ident = w2[:, 768:896]
            xt = pp.tile([128, 8 * T], f16, tag="xt", name="xt")
            xt3 = xt.rearrange("p (c t) -> p c t", c=8)
            xt5 = xt.rearrange("p (c t4 two par tb) -> p c t4 two par tb",
                               c=8, t4=4, two=2, par=2, tb=128)
            ktvt = [pp.tile([128, 512], f16, tag=f"ktvt{t}", name=f"ktvt{t}")
                    for t in range(4)]
            QT = pp.tile([64, 1024], f16, tag="qt", name="qt")
            vnat = [pp.tile([128, 260], bf16, tag=f"vnat{t}", name=f"vnat{t}")
                    for t in range(4)]
            vnat3 = [v.rearrange("p (k c) -> p k c", k=4) for v in vnat]
            yt = pp.tile([128, 512], f32, tag="yt", name="yt")

            # ---- PE p-state warmup: garbage matmuls during DMA window ----
            nc.vector.memset(warm, 0.0)
            for d in range(NDUMMY):
                ps = pa.tile([128, 512], f32, tag="pa", name="kvps")
                nc.tensor.matmul(ps[:, 0:128], warm, warm, start=True, stop=True)

            # vnat ones-columns (col 64 of each 65-wide block)
            for t in range(4):
                nc.vector.memset(vnat[t], 1.0)

            # ---- input DMAs: big merged transfers; chunk 0 split in halves ----
            nc.sync.dma_start(wkv, wkv_d)
            xt_dr = xt_d.rearrange("c p t -> p c t")
            nc.sync.dma_start(xt3[:, :, 0:256], xt_dr[:, :, 0:256])
            nc.sync.dma_start(xt3[:, :, 256:512], xt_dr[:, :, 256:512])
            nc.sync.dma_start(w2, w2_d)
            for hh in range(2, 8):
                nc.sync.dma_start(
                    xt3[:, :, 256 * hh:256 * (hh + 1)],
                    xt_dr[:, :, 256 * hh:256 * (hh + 1)],
                )

            def attn(i):
                    # ---- attention for q-tile i ----
                    # Chunks are exp'd in pair-groups sharing one [128,1024]
                    # PSUM tile (one ACT instruction per pair).
                    nch = NCH[i]
                    nbd = 2 + 2 * (i % 2)
                    groups = [tuple(c for c in (g, g + 1) if c < nch)
                              for g in range(0, nch, 2)]
                    Es = []   # (E tile, [(col_off, chunk, k), ...])
                    for cs in groups:
                        sps = pb.tile([128, 1024], f32, tag="st", name="st")
                        col = 0
                        blocks = []
                        for c in cs:
                            if c == nch - 1:
                                for k in range(nbd - 2):
                                    nc.tensor.matmul(
                                        sps[:, col + 128 * k:col + 128 * (k + 1)],
                                        ktvt[c][0:64, 128 * k:128 * (k + 1)],
                                        QT[:, 128 * i:128 * (i + 1)],
                                        start=True, stop=True,
                                    )
                                nc.tensor.matmul(
                                    sps[:, col + 128 * (nbd - 2):col + 128 * nbd],
                                    ident, dmask,
                                    start=True, stop=False,
                                )
                                for k in (nbd - 2, nbd - 1):
                                    nc.tensor.matmul(
                                        sps[:, col + 128 * k:col + 128 * (k + 1)],
                                        ktvt[c][0:64, 128 * k:128 * (k + 1)],
                                        QT[:, 128 * i:128 * (i + 1)],
                                        start=False, stop=(k == nbd - 1),
                                        skip_group_check=True,
                                    )
                                nb = nbd
                            else:
                                nb = 4
                                for k in range(4):
                                    nc.tensor.matmul(
                                        sps[:, col + 128 * k:col + 128 * (k + 1)],
                                        ktvt[c][0:64, 128 * k:128 * (k + 1)],
                                        QT[:, 128 * i:128 * (i + 1)],
                                        start=True, stop=True,
                                    )
                            blocks += [(col + 128 * k, c, k) for k in range(nb)]
                            col += 128 * nb
                        E = ep.tile([128, 1024], bf16, tag="E", name="E")
                        nc.scalar.activation(E[:, 0:col], sps[:, 0:col], Exp)
                        Es.append((E, blocks))
                    po = pc.tile([128, 128], f32, tag="po", name="po")
                    nmm = sum(len(bl) for _, bl in Es)
                    m = 0
                    for E, bl in Es:
                        for off, c, k in bl:
                            nc.tensor.matmul(
                                po[:, 0:65],
                                E[:, off:off + 128],
                                vnat3[c][:, k, :],
                                start=(m == 0), stop=(m == nmm - 1),
                            )
                            m += 1
                    rinv = smp.tile([128, 1], f32, tag="rinv", name="rinv")
                    nc.vector.reciprocal(rinv, po[:, 64:65])
                    nc.vector.tensor_scalar_mul(yt[:, 64 * i:64 * (i + 1)],
                                                po[:, 0:64], rinv[:, 0:1])

            for tch in range(4):
                t0 = 512 * tch
                ps = pa.tile([128, 512], f32, tag="pa", name="kvps")
                qps = pa.tile([128, 512], f32, tag="pa", name="qps")
                vps = pa.tile([128, 512], f16, tag="pav", name="vps", bufs=1)
                vps3 = vps.rearrange("p (k c) -> p k c", k=8)
                if tch < 3:
                    # chunk-level schedule; ktvt copied per half (half a
                    # copies during the inter-slab gap) and for tch>0 the Q
                    # projection is emitted before the second KV half-pass,
                    # so the even tile's diagonal (half-a keys) unlocks
                    # right after slab b lands instead of a full copy later.
                    def q_proj():
                        for cj in range(8):
                            nc.tensor.matmul(
                                qps[0:64, 0:256],
                                wq[:, 64 * cj:64 * (cj + 1)],
                                xt5[:, cj, tch, :, 0, :],
                                start=(cj == 0), stop=(cj == 7),
                            )
                        if tch == 0:  # ACT idle early; DVE busy with copies
                            nc.scalar.activation(QT[:, 0:256],
                                                 qps[0:64, 0:256], Copy)
                        else:
                            nc.vector.tensor_copy(
                                QT[:, 256 * tch:256 * (tch + 1)],
                                qps[0:64, 0:256])
                    for h in range(2):
                        for cj in range(8):
                            nc.tensor.matmul(
                                ps[:, 256 * h:256 * (h + 1)],
                                wkv[:, 128 * cj:128 * (cj + 1)],
                                xt3[:, cj, t0 + 256 * h:t0 + 256 * (h + 1)],
                                start=(cj == 0), stop=(cj == 7),
                            )
                        nc.vector.tensor_copy(
                            ktvt[tch][:, 256 * h:256 * (h + 1)],
                            ps[:, 256 * h:256 * (h + 1)])
                        if h == 0 and tch > 0:
                            q_proj()
                        for k in (2 * h, 2 * h + 1):
                            nc.tensor.transpose(
                                vps[:, 64 * k:64 * (k + 1)],
                                ktvt[tch][64:128, 128 * k:128 * (k + 1)],
                                ident[64:128, 64:128],
                            )
                        nc.vector.tensor_copy(
                            vnat3[tch][:, 2 * h:2 * h + 2, 0:64],
                            vps3[:, 2 * h:2 * h + 2, :])
                    if tch == 0:
                        q_proj()
                    attn(2 * tch)
                    attn(2 * tch + 1)
                else:
                    # last chunk per half-slab: tile 6 (diag = blocks 0,1)
                    # is fully ready after half a; only tile 7 waits for
                    # the final half-slab b -> short kernel tail.
                    for h in range(2):
                        i = 6 + h
                        for cj in range(8):
                            nc.tensor.matmul(
                                ps[:, 256 * h:256 * (h + 1)],
                                wkv[:, 128 * cj:128 * (cj + 1)],
                                xt3[:, cj, t0 + 256 * h:t0 + 256 * (h + 1)],
                                start=(cj == 0), stop=(cj == 7),
                            )
                        nc.vector.tensor_copy(
                            ktvt[tch][:, 256 * h:256 * (h + 1)],
                            ps[:, 256 * h:256 * (h + 1)])
                        for cj in range(8):
                            nc.tensor.matmul(
                                qps[0:64, 128 * h:128 * (h + 1)],
                                wq[:, 64 * cj:64 * (cj + 1)],
                                xt5[:, cj, tch, h, 0, :],
                                start=(cj == 0), stop=(cj == 7),
                            )
                        nc.vector.tensor_copy(QT[:, 128 * i:128 * (i + 1)],
                                              qps[0:64, 128 * h:128 * (h + 1)])
                        for k in (2 * h, 2 * h + 1):
                            nc.tensor.transpose(
                                vps[:, 64 * k:64 * (k + 1)],
                                ktvt[tch][64:128, 128 * k:128 * (k + 1)],
                                ident[64:128, 64:128],
                            )
                        nc.vector.tensor_copy(
                            vnat3[tch][:, 2 * h:2 * h + 2, 0:64],
                            vps3[:, 2 * h:2 * h + 2, :])
                        attn(i)
                nc.sync.dma_start(y_d[:, 128 * tch:128 * (tch + 1)],
                                  yt[:, 128 * tch:128 * (tch + 1)])

    nc.compile()
    return nc


def _host_inputs(x, Wq, Wk, Wv):
    """Per-core input maps. Core c = 2*b + j."""
    f16 = np.float16
    wkv = np.empty((128, 1024), f16)
    Wv8 = Wv * 0.125
    for cj in range(8):
        wkv[:, 128 * cj:128 * cj + 64] = Wk[128 * cj:128 * (cj + 1), :]
        wkv[:, 128 * cj + 64:128 * (cj + 1)] = Wv8[128 * cj:128 * (cj + 1), :]
    wq = np.empty((128, 512), f16)
    for cj in range(8):
        wq[:, 64 * cj:64 * (cj + 1)] = Wq[128 * cj:128 * (cj + 1), :]
    tri = np.zeros((128, 128), np.float32)
    tri[np.arange(128)[:, None] > np.arange(128)[None, :]] = NEG
    w2 = [np.zeros((128, 896), f16) for _ in range(2)]
    for j in range(2):
        w2[j][:, 0:512] = wq
        w2[j][:, 512:640] = tri          # diag block of preload pair
        w2[j][:, 640:768] = NEG if j == 0 else 0.0  # past-diag block
        w2[j][:, 768:896] = np.eye(128, dtype=f16)

    in_maps = []
    for core in range(8):
        b, j = divmod(core, 2)
        xT = x[b].T.astype(f16)          # [1024, 2048]
        if j == 1:
            # swap adjacent 128-col blocks so q-cols sit at even positions
            xT = xT.reshape(1024, 8, 2, 128)[:, :, ::-1, :].reshape(1024, 2048)
        in_maps.append({
            "xt": np.ascontiguousarray(xT).reshape(8, 128, T),
            "wkv": wkv,
            "w2": w2[j],
        })
    return in_maps


def kernel(x, Wq, Wk, Wv):
    from concourse.bass_utils import run_bass_kernel_spmd

    x = np.asarray(x, dtype=np.float32)
    Wq = np.asarray(Wq, dtype=np.float32)
    Wk = np.asarray(Wk, dtype=np.float32)
    Wv = np.asarray(Wv, dtype=np.float32)

    if "nc" not in _CACHE:
        _CACHE["nc"] = _build()
    nc = _CACHE["nc"]

    in_maps = _host_inputs(x, Wq, Wk, Wv)
    res = run_bass_kernel_spmd(nc, in_maps, core_ids=list(range(8)))
    out = np.empty((B, T, DK), dtype=np.float32)
    for core in range(8):
        b, j = divmod(core, 2)
        yloc = res.results[core]["y"]    # [128, 512]
        for i in range(8):
            g = 2 * i + j
            out[b, 128 * g:128 * (g + 1), :] = yloc[:, 64 * i:64 * (i + 1)]
    return out


# revision 65
# speedup vs baseline: 1.0230x; 1.0230x over previous
"""Causal single-head attention block on 8 TRN2 NeuronCores.

Reference: Q=x@Wq, K=x@Wk, V=x@Wv; S=Q@K^T (no pre-softmax scaling);
causal mask; P=softmax(S); out=(P@V)/sqrt(64).
Shapes: x [4, 2048, 1024] f32, W* [1024, 64] f32 -> out [4, 2048, 64].

Sharding: 8 cores = 4 batches x 2 interleaved query-tile sets.
Core (b, j) handles global 128-row query tiles {2i+j : i=0..7}.

Key design points (vs naive):
  * x is transposed and cast to fp16 on the host; the device loads xT
    directly (no on-chip transposes of x, no duplicate xq load).
  * For SPMD uniformity, j=1 cores get adjacent 128-column blocks of xT
    swapped so query columns sit at even block positions for all cores.
    Key order within a chunk changes, which is harmless (attention sums
    over keys); the causal mask data (per-core) accounts for it.
  * Attention computed transposed: St[t,q] = K @ Q^T per 128-key block,
    so exp() output E already has keys on partitions -> AV matmul needs
    no transposes at all. Rowsum obtained for free via an extra ones
    column appended to V-natural (col 64), accumulated in the same PSUM.
  * Causal mask is preloaded into PSUM with an identity matmul (PE),
    covering the last two 128-key blocks of the diagonal chunk; block
    counts are 2 (even tiles) / 4 (odd tiles) for every core.
  * fp16 for x/W/Q/K/S path, bf16 for E/V (exp range needs bf16);
    1/sqrt(64)=0.125 folded into Wv on the host. rel_err ~5e-3.
  * Input DMAs are merged into a few large transfers (wkv; rest-of-
    weights; 8 half-chunk loads of [128, 8, 256]) to amortize the
    ~650ns/DMA DGE issue overhead while still pipelining: each K|V
    projection runs in two half passes as its halves land.
  * Chunks are exp'd in pair-groups sharing one [128,1024] PSUM tile:
    one ACT instruction per pair halves the ACT per-inst overhead
    (ACT/exp is the critical engine in the back half).
  * ~30 dummy PE matmuls at t=0 warm the PE p-state during the
    initial DMA window so real matmuls run at full clock.
"""

import sys

import numpy as np

try:  # concourse ships in the TRN container; fall back to its known path
    import concourse  # noqa: F401
except ImportError:
    sys.path.insert(0, "/opt/trn_rl_repo")

B, T, C, DK = 4, 2048, 1024, 64
NCH = [1, 1, 2, 2, 3, 3, 4, 4]   # 512-key chunks per local q-tile (both j)
NDUMMY = 30                       # PE p-state warmup matmuls
NEG = -30000.0                    # fp16-safe mask value

_CACHE = {}


def _build():
    import concourse.bacc as bacc
    import concourse.tile as tile
    import concourse.mybir as mybir

    f32 = mybir.dt.float32
    f16 = mybir.dt.float16
    bf16 = mybir.dt.bfloat16
    Exp = mybir.ActivationFunctionType.Exp
    Copy = mybir.ActivationFunctionType.Copy

    nc = bacc.Bacc("TRN2", target_bir_lowering=False, debug=False,
                   enable_asserts=False, num_devices=8)

    xt_d = nc.dram_tensor("xt", [8, 128, T], f16, kind="ExternalInput").ap()
    wkv_d = nc.dram_tensor("wkv", [128, 1024], f16, kind="ExternalInput").ap()
    w2_d = nc.dram_tensor("w2", [128, 896], f16, kind="ExternalInput").ap()
    y_d = nc.dram_tensor("y", [128, 512], f32, kind="ExternalOutput").ap()

    with tile.TileContext(nc) as tc:
        with (
            tc.tile_pool(name="persist", bufs=1) as pp,
            tc.tile_pool(name="epool", bufs=8) as ep,
            tc.tile_pool(name="small", bufs=4) as smp,
            tc.tile_pool(name="pa", bufs=2, space="PSUM") as pa,
            tc.tile_pool(name="pb", bufs=2, space="PSUM") as pb,
            tc.tile_pool(name="pc", bufs=1, space="PSUM") as pc,
        ):
            warm = pp.tile([128, 128], f16, tag="warm", name="warm")
            wkv = pp.tile([128, 1024], f16, tag="wkv", name="wkv")
            w2 = pp.tile([128, 896], f16, tag="w2", name="w2")
            wq = w2[:, 0:512]
            dmask = w2[:, 512:768]
            ident = w2[:, 768:896]
            xt = pp.tile([128, 8 * T], f16, tag="xt", name="xt")
            xt3 = xt.rearrange("p (c t) -> p c t", c=8)
            xt5 = xt.rearrange("p (c t4 two par tb) -> p c t4 two par tb",
                               c=8, t4=4, two=2, par=2, tb=128)
            ktvt = [pp.tile([128, 512], f16, tag=f"ktvt{t}", name=f"ktvt{t}")
                    for t in range(4)]
            QT = pp.tile([64, 1024], f16, tag="qt", name="qt")
            vnat = [pp.tile([128, 260], bf16, tag=f"vnat{t}", name=f"vnat{t}")
                    for t in range(4)]
            vnat3 = [v.rearrange("p (k c) -> p k c", k=4) for v in vnat]
            yt = pp.tile([128, 512], f32, tag="yt", name="yt")

            # ---- PE p-state warmup: garbage matmuls during DMA window ----
            nc.vector.memset(warm, 0.0)
            for d in range(NDUMMY):
                ps = pa.tile([128, 512], f32, tag="pa", name="kvps")
                nc.tensor.matmul(ps[:, 0:128], warm, warm, start=True, stop=True)

            # vnat ones-columns (col 64 of each 65-wide block)
            for t in range(4):
                nc.vector.memset(vnat[t], 1.0)

            # ---- input DMAs: big merged transfers; chunk 0 split in halves ----
            nc.sync.dma_start(wkv, wkv_d)
            xt_dr = xt_d.rearrange("c p t -> p c t")
            nc.sync.dma_start(xt3[:, :, 0:256], xt_dr[:, :, 0:256])
            nc.sync.dma_start(xt3[:, :, 256:512], xt_dr[:, :, 256:512])
            nc.sync.dma_start(w2, w2_d)
            for hh in range(2, 8):
                nc.sync.dma_start(
                    xt3[:, :, 256 * hh:256 * (hh + 1)],
                    xt_dr[:, :, 256 * hh:256 * (hh + 1)],
                )

            def attn(i):
                    # ---- attention for q-tile i ----
                    # Chunks are exp'd in pair-groups sharing one [128,1024]
                    # PSUM tile (one ACT instruction per pair).
                    nch = NCH[i]
                    nbd = 2 + 2 * (i % 2)
                    groups = [tuple(c for c in (g, g + 1) if c < nch)
                              for g in range(0, nch, 2)]
                    Es = []   # (E tile, [(col_off, chunk, k), ...])
                    for cs in groups:
                        sps = pb.tile([128, 1024], f32, tag="st", name="st")
                        col = 0
                        blocks = []
                        for c in cs:
                            if c == nch - 1:
                                for k in range(nbd - 2):
                                    nc.tensor.matmul(
                                        sps[:, col + 128 * k:col + 128 * (k + 1)],
                                        ktvt[c][0:64, 128 * k:128 * (k + 1)],
                                        QT[:, 128 * i:128 * (i + 1)],
                                        start=True, stop=True,
                                    )
                                nc.tensor.matmul(
                                    sps[:, col + 128 * (nbd - 2):col + 128 * nbd],
                                    ident, dmask,
                                    start=True, stop=False,
                                )
                                for k in (nbd - 2, nbd - 1):
                                    nc.tensor.matmul(
                                        sps[:, col + 128 * k:col + 128 * (k + 1)],
                                        ktvt[c][0:64, 128 * k:128 * (k + 1)],
                                        QT[:, 128 * i:128 * (i + 1)],
                                        start=False, stop=(k == nbd - 1),
                                        skip_group_check=True,
                                    )
                                nb = nbd
                            else:
                                nb = 4
                                for k in range(4):
                                    nc.tensor.matmul(
                                        sps[:, col + 128 * k:col + 128 * (k + 1)],
                                        ktvt[c][0:64, 128 * k:128 * (k + 1)],
                                        QT[:, 128 * i:128 * (i + 1)],
                                        start=True, stop=True,
                                    )
                            blocks += [(col + 128 * k, c, k) for k in range(nb)]
                            col += 128 * nb
                        E = ep.tile([128, 1024], bf16, tag="E", name="E")
                        nc.scalar.activation(E[:, 0:col], sps[:, 0:col], Exp)
                        Es.append((E, blocks))
                    po = pc.tile([128, 128], f32, tag="po", name="po")
                    nmm = sum(len(bl) for _, bl in Es)
                    m = 0
                    for E, bl in Es:
                        for off, c, k in bl:
                            nc.tensor.matmul(
                                po[:, 0:65],
                                E[:, off:off + 128],
                                vnat3[c][:, k, :],
                                start=(m == 0), stop=(m == nmm - 1),
                            )
                            m += 1
                    rinv = smp.tile([128, 1], f32, tag="rinv", name="rinv")
                    nc.vector.reciprocal(rinv, po[:, 64:65])
                    nc.vector.tensor_scalar_mul(yt[:, 64 * i:64 * (i + 1)],
                                                po[:, 0:64], rinv[:, 0:1])

            for tch in range(4):
                t0 = 512 * tch
                ps = pa.tile([128, 512], f32, tag="pa", name="kvps")
                qps = pa.tile([128, 512], f32, tag="pa", name="qps")
                vps = pa.tile([128, 512], f16, tag="pav", name="vps", bufs=1)
                vps3 = vps.rearrange("p (k c) -> p k c", k=8)
                if tch < 3:
                    # chunk-level schedule; ktvt copied per half (half a
                    # copies during the inter-slab gap) and for tch>0 the Q
                    # projection is emitted before the second KV half-pass,
                    # so the even tile's diagonal (half-a keys) unlocks
                    # right after slab b lands instead of a full copy later.
                    def q_proj():
                        for cj in range(8):
                            nc.tensor.matmul(
                                qps[0:64, 0:256],
                                wq[:, 64 * cj:64 * (cj + 1)],
                                xt5[:, cj, tch, :, 0, :],
                                start=(cj == 0), stop=(cj == 7),
                            )
                        if tch == 0:  # ACT idle early; DVE busy with copies
                            nc.scalar.activation(QT[:, 0:256],
                                                 qps[0:64, 0:256], Copy)
                        else:
                            nc.vector.tensor_copy(
                                QT[:, 256 * tch:256 * (tch + 1)],
                                qps[0:64, 0:256])
                    for h in range(2):
                        for cj in range(8):
                            nc.tensor.matmul(
                                ps[:, 256 * h:256 * (h + 1)],
                                wkv[:, 128 * cj:128 * (cj + 1)],
                                xt3[:, cj, t0 + 256 * h:t0 + 256 * (h + 1)],
                                start=(cj == 0), stop=(cj == 7),
                            )
                        nc.vector.tensor_copy(
                            ktvt[tch][:, 256 * h:256 * (h + 1)],
                            ps[:, 256 * h:256 * (h + 1)])
                        if h == 0 and tch > 0:
                            q_proj()
                    if tch == 0:
                        q_proj()
                    for k in range(4):
                        nc.tensor.transpose(
                            vps[:, 64 * k:64 * (k + 1)],
                            ktvt[tch][64:128, 128 * k:128 * (k + 1)],
                            ident[64:128, 64:128],
                        )
                    nc.vector.tensor_copy(vnat3[tch][:, :, 0:64],
                                          vps3[:, 0:4, :])
                    attn(2 * tch)
                    attn(2 * tch + 1)
                else:
                    # last chunk per half-slab: tile 6 (diag = blocks 0,1)
                    # is fully ready after half a; only tile 7 waits for
                    # the final half-slab b -> short kernel tail.
                    for h in range(2):
                        i = 6 + h
                        for cj in range(8):
                            nc.tensor.matmul(
                                ps[:, 256 * h:256 * (h + 1)],
                                wkv[:, 128 * cj:128 * (cj + 1)],
                                xt3[:, cj, t0 + 256 * h:t0 + 256 * (h + 1)],
                                start=(cj == 0), stop=(cj == 7),
                            )
                        nc.vector.tensor_copy(
                            ktvt[tch][:, 256 * h:256 * (h + 1)],
                            ps[:, 256 * h:256 * (h + 1)])
                        for cj in range(8):
                            nc.tensor.matmul(
                                qps[0:64, 128 * h:128 * (h + 1)],
                                wq[:, 64 * cj:64 * (cj + 1)],
                                xt5[:, cj, tch, h, 0, :],
                                start=(cj == 0), stop=(cj == 7),
                            )
                        nc.vector.tensor_copy(QT[:, 128 * i:128 * (i + 1)],
                                              qps[0:64, 128 * h:128 * (h + 1)])
                        for k in (2 * h, 2 * h + 1):
                            nc.tensor.transpose(
                                vps[:, 64 * k:64 * (k + 1)],
                                ktvt[tch][64:128, 128 * k:128 * (k + 1)],
                                ident[64:128, 64:128],
                            )
                        nc.vector.tensor_copy(
                            vnat3[tch][:, 2 * h:2 * h + 2, 0:64],
                            vps3[:, 2 * h:2 * h + 2, :])
                        attn(i)
                nc.sync.dma_start(y_d[:, 128 * tch:128 * (tch + 1)],
                                  yt[:, 128 * tch:128 * (tch + 1)])

    nc.compile()
    return nc


def _host_inputs(x, Wq, Wk, Wv):
    """Per-core input maps. Core c = 2*b + j."""
    f16 = np.float16
    wkv = np.empty((128, 1024), f16)
    Wv8 = Wv * 0.125
    for cj in range(8):
        wkv[:, 128 * cj:128 * cj + 64] = Wk[128 * cj:128 * (cj + 1), :]
        wkv[:, 128 * cj + 64:128 * (cj + 1)] = Wv8[128 * cj:128 * (cj + 1), :]
    wq = np.empty((128, 512), f16)
    for cj in range(8):
        wq[:, 64 * cj:64 * (cj + 1)] = Wq[128 * cj:128 * (cj + 1), :]
    tri = np.zeros((128, 128), np.float32)
    tri[np.arange(128)[:, None] > np.arange(128)[None, :]] = NEG
    w2 = [np.zeros((128, 896), f16) for _ in range(2)]
    for j in range(2):
        w2[j][:, 0:512] = wq
        w2[j][:, 512:640] = tri          # diag block of preload pair
        w2[j][:, 640:768] = NEG if j == 0 else 0.0  # past-diag block
        w2[j][:, 768:896] = np.eye(128, dtype=f16)

    in_maps = []
    for core in range(8):
        b, j = divmod(core, 2)
        xT = x[b].T.astype(f16)          # [1024, 2048]
        if j == 1:
            # swap adjacent 128-col blocks so q-cols sit at even positions
            xT = xT.reshape(1024, 8, 2, 128)[:, :, ::-1, :].reshape(1024, 2048)
        in_maps.append({
            "xt": np.ascontiguousarray(xT).reshape(8, 128, T),
            "wkv": wkv,
            "w2": w2[j],
        })
    return in_maps


def kernel(x, Wq, Wk, Wv):
    from concourse.bass_utils import run_bass_kernel_spmd

    x = np.asarray(x, dtype=np.float32)
    Wq = np.asarray(Wq, dtype=np.float32)
    Wk = np.asarray(Wk, dtype=np.float32)
    Wv = np.asarray(Wv, dtype=np.float32)

    if "nc" not in _CACHE:
        _CACHE["nc"] = _build()
    nc = _CACHE["nc"]

    in_maps = _host_inputs(x, Wq, Wk, Wv)
    res = run_bass_kernel_spmd(nc, in_maps, core_ids=list(range(8)))
    out = np.empty((B, T, DK), dtype=np.float32)
    for core in range(8):
        b, j = divmod(core, 2)
        yloc = res.results[core]["y"]    # [128, 512]
        for i in range(8):
            g = 2 * i + j
            out[b, 128 * g:128 * (g + 1), :] = yloc[:, 64 * i:64 * (i + 1)]
    return out
